# revision 1
# baseline (speedup 1.0000x reference)
"""MultiHeadAttention TRN2 Bass kernel (v5).

Problem: S=2048, B=2, H=16, d_k=64, D=1024, fp32.
  q = query @ Wq.T + bq ; k = key @ Wk.T + bk ; v = value @ Wv.T + bv
  score = einsum('qbhd,kbhd->qkbh', q, k) / 8 ; attn = softmax(score, axis=k)
  out = einsum('qkbh,kbhd->qbhd', attn, v) -> reshape -> @ Wo.T + bo

Sharding (8 cores): core c handles batch b = c//4 and heads [4*(c%4), 4*(c%4)+4)
(tensor-parallel along the head dimension). The device computes the QKV
projections and the attention (scores -> exp -> PV with a fused ones-column
denominator). Each core returns its raw PV numerators [256, 2048] plus the
16 softmax denominator rows; the gather/unshard step divides and applies
the output projection while it sums the per-core partials and bias terms.

Device schedule (see v3/v4 notes in git history):
  - All matmul operands fp16; fp32 PSUM accumulate (rel err ~8e-4 vs 2e-2).
  - Scalar engine runs ONLY the 128 exp activations [128,1024] — the hard
    ~.14ms/core floor everything else hides under.
  - Host passes tensors pre-packed in SBUF layout (multi-KB descriptors);
    x tiles stream over the Sync HWDGE queue in just-in-time order, weights
    over the Activation HWDGE queue at kernel start.
  - A dozen dummy matmuls ramp the PE DVFS p-state during the DMA prefix.
  - PV matmuls trail the score matmuls by 2 kb-blocks; projections drain as
    keyed PE filler between attention matmuls (forced drains keep producers
    ahead of consumers in the in-order engine queues).
"""

import os

os.environ.setdefault("MYCRO_LOCAL_CACHE", "1")

import numpy as np

import concourse.bass as bass
import concourse.tile as tile
from concourse import bacc, bass_utils, mybir


def _install_ntff_hook():
    """Provide antenv.axon_hooks when the image lacks it, so trace=True can
    capture NTFF profiles through the axon tunnel. Degrades silently."""
    import contextlib
    import ctypes
    import sys

    if "antenv.axon_hooks" in sys.modules:
        return
    so_path = "/opt/axon/libaxon_pjrt.so"
    if not os.path.exists(so_path):
        return
    try:
        lib = ctypes.CDLL(so_path)
        if not hasattr(lib, "axon_start_nrt_profile"):
            return
        lib.axon_start_nrt_profile.argtypes = [
            ctypes.POINTER(ctypes.c_int64),
            ctypes.c_size_t,
        ]
        lib.axon_start_nrt_profile.restype = ctypes.c_int64
        lib.axon_stop_nrt_profile.argtypes = [ctypes.c_char_p]
        lib.axon_stop_nrt_profile.restype = ctypes.c_int64

        @contextlib.contextmanager
        def _hook(output_dir, device_ids):
            import jax

            jax.devices()
            if device_ids:
                ids = (ctypes.c_int64 * len(device_ids))(*device_ids)
                rc = lib.axon_start_nrt_profile(ids, len(device_ids))
            else:
                rc = lib.axon_start_nrt_profile(None, 0)
            if rc != 0:
                raise RuntimeError(f"axon_start_nrt_profile rc={rc}")
            try:
                yield
            finally:
                n = lib.axon_stop_nrt_profile(str(output_dir).encode())
                print(f"ntff profile: {n} file(s) -> {output_dir}")

        import types

        mod = types.ModuleType("antenv.axon_hooks")
        mod.get_axon_ntff_profile_hook = lambda: _hook
        mod.set_axon_ntff_profile_hook = lambda h: None
        sys.modules["antenv.axon_hooks"] = mod
    except Exception:
        pass


_install_ntff_hook()

F32 = mybir.dt.float32
FP16 = mybir.dt.float16
AF = mybir.ActivationFunctionType

S = 2048          # sequence length
B = 2             # batch
H = 16            # total heads
DK = 64           # head dim
D = 1024          # model dim
NCORES = 8
HL = H // (NCORES // B)   # heads per core = 4
HC = HL * DK              # head cols per core = 256
T = S                     # tokens per core (one batch element)
P = 128
QB = 512                  # q block (matmul free dim)
NKB = T // P              # 16 k blocks
NQB = T // QB             # 4 q blocks
NKC = D // P              # 8 contraction chunks for projections
VW = DK + 1               # 65: head value cols + ones column


def build_module():
    nc = bacc.Bacc("TRN2", target_bir_lowering=False, debug=False)

    # Inputs pre-packed by the host in SBUF layout (partition-major,
    # contiguous per partition -> multi-KB DMA descriptors).
    xq = nc.dram_tensor("xq", [NQB, P, NKC, QB], FP16, kind="ExternalInput").ap()
    xk = nc.dram_tensor("xk", [NQB, P, NKC, QB], FP16, kind="ExternalInput").ap()
    xv = nc.dram_tensor("xv", [NQB, P, NKC, QB], FP16, kind="ExternalInput").ap()
    wq = nc.dram_tensor("wq", [2, P, NKC, P], FP16, kind="ExternalInput").ap()
    wk = nc.dram_tensor("wk", [2, P, NKC, P], FP16, kind="ExternalInput").ap()
    wv = nc.dram_tensor("wv", [P, NKC, HC], FP16, kind="ExternalInput").ap()
    bqv = nc.dram_tensor("bqv", [P, HC // P], F32, kind="ExternalInput").ap()
    bkv = nc.dram_tensor("bkv", [P, HC // P], F32, kind="ExternalInput").ap()
    # raw attention numerators [m, qb, 128, 512] and denominators
    # (flat [ (4qb+h)*512 + col ] on one partition: engines may not write
    # single-partition tiles at arbitrary partition offsets)
    ac = nc.dram_tensor("ac", [2, NQB, P, QB], FP16, kind="ExternalOutput").ap()
    dn = nc.dram_tensor("dn", [NQB * HL * QB], F32, kind="ExternalOutput").ap()

    with tile.TileContext(nc) as tc:
        kernel_body(tc, xq, xk, xv, wq, wk, wv, bqv, bkv, ac, dn)

    nc.compile()
    return nc


def kernel_body(tc, xq, xk, xv, wq, wk, wv, bqv, bkv, ac, dn):
    nc = tc.nc

    with (
        tc.tile_pool(name="attn", bufs=8) as attn_pool,
        tc.tile_pool(name="consts", bufs=1) as consts,
        tc.tile_pool(name="persist", bufs=1) as persist,
        tc.tile_pool(name="late", bufs=1) as late,
        tc.tile_pool(name="ps_mm", bufs=2, space="PSUM") as ps_mm,
        tc.tile_pool(name="ps_sc", bufs=2, space="PSUM") as ps_sc,
        tc.tile_pool(name="ps_pv", bufs=2, space="PSUM") as ps_pv,
    ):
        # at tiles first: the Activation engine's SBUF write latency grows
        # with address, and the 128 exps are the kernel's critical path.
        at_tiles = [
            attn_pool.tile([P, 2 * QB], FP16, tag="at", name=f"at_{i}")
            for i in range(8)
        ]
        # ---------------- PE warm-up (ramps the DVFS p-state) --------------
        dummy = consts.tile([1, QB], FP16)
        nc.vector.memset(dummy, 1.0)
        warm_ps = ps_mm.tile([DK, QB], F32, tag="mm", name="warm")
        for _ in range(6):
            nc.tensor.matmul(
                warm_ps, lhsT=dummy[:, :DK], rhs=dummy, start=True, stop=True
            )

        # ---------------- inputs: three parallel DMA queues ----------------
        # Each hardware/software queue streams ~1MB/8us, so K, V and Q
        # traffic ride separate queues, ordered just-in-time. wk/wq are
        # split per 128-col m-chunk so stage A waits on only 0.25MB.
        xk_t = [persist.tile([P, NKC, QB], FP16, name=f"xk{tb}") for tb in range(4)]
        xv_t = [persist.tile([P, NKC, QB], FP16, name=f"xv{tb}") for tb in range(4)]
        xq_t = [persist.tile([P, NKC, QB], FP16, name=f"xq{tb}") for tb in range(4)]
        wk_s = [consts.tile([P, NKC, P], FP16, name=f"wk_s{m}") for m in range(2)]
        wq_s = [consts.tile([P, NKC, P], FP16, name=f"wq_s{m}") for m in range(2)]
        wv_s = consts.tile([P, NKC, HC], FP16)
        bk_s = consts.tile([P, HC // P], F32)
        bq_s = consts.tile([P, HC // P], F32)

        # Sync queue: all x inputs just-in-time (one queue sustains ~1MB/9us;
        # the ACT queue moves xq0 slower than sync even under contention)
        for dst, src, tb in (
            (xk_t[0], xk, 0), (xq_t[0], xq, 0),
            (xv_t[0], xv, 0), (xv_t[1], xv, 1),
            (xk_t[1], xk, 1), (xv_t[2], xv, 2),
            (xk_t[2], xk, 2), (xv_t[3], xv, 3),
            (xk_t[3], xk, 3),
            (xq_t[1], xq, 1), (xq_t[2], xq, 2), (xq_t[3], xq, 3),
        ):
            nc.sync.dma_start(dst, src[tb])
        # Activation HWDGE queue: the small weight/bias chunks in parallel
        nc.scalar.dma_start(wk_s[0], wk[0])
        nc.scalar.dma_start(wq_s[0], wq[0])
        nc.scalar.dma_start(wk_s[1], wk[1])
        nc.scalar.dma_start(wq_s[1], wq[1])
        nc.scalar.dma_start(bq_s, bqv)
        nc.scalar.dma_start(bk_s, bkv)
        # GpSimd queue: only the V weights (bulk x traffic on this software
        # queue slows the whole DMA fabric — measured +15us)
        nc.gpsimd.dma_start(wv_s, wv)

        ones_f32 = consts.tile([P, DK], F32)
        nc.vector.memset(ones_f32, 1.0)

        # ---------------- persistent activations ----------------
        QT = [persist.tile([P, T], FP16, name=f"QT{m}") for m in range(2)]
        KT = [persist.tile([P, T], FP16, name=f"KT{m}") for m in range(2)]
        V = persist.tile([P, NKB, HL * VW], FP16, name="V")

        # ones columns of V (denominator trick); also warms the act table
        nc.scalar.activation(
            V.rearrange("p t (h c) -> p t h c", c=VW)[:, :, :, DK],
            ones_f32[:, : NKB * HL].rearrange("p (t h) -> p t h", h=HL),
            AF.Copy,
        )

        # ---------------- projection emitters (merged drain units) ---------
        def proj_qk_direct(xt, w_s, b_s, dst, m, tb):
            ps = ps_mm.tile([P, QB], F32, tag="mm", name=f"pd_{dst[0].name}{m}{tb}")
            for kc in range(NKC):
                nc.tensor.matmul(
                    ps,
                    lhsT=w_s[m][:, kc, :],
                    rhs=xt[:, kc, :],
                    start=(kc == 0),
                    stop=(kc == NKC - 1),
                )
            nc.vector.tensor_scalar_add(
                dst[m][:, tb * QB : (tb + 1) * QB], ps, b_s[:, m : m + 1]
            )

        def proj_qk_units(xt, w_s, b_s, dst, m, tb, tag):
            # 8 units: [alloc+mm0], mm1..mm6, [mm7+evac]
            units = []
            st = {}
            for kc in range(NKC):

                def mk(kc=kc, st=st):
                    if kc == 0:
                        st["ps"] = ps_mm.tile(
                            [P, QB], F32, tag="mm", name=f"pz_{tag}{m}{tb}"
                        )
                    nc.tensor.matmul(
                        st["ps"],
                        lhsT=w_s[m][:, kc, :],
                        rhs=xt[:, kc, :],
                        start=(kc == 0),
                        stop=(kc == NKC - 1),
                    )
                    if kc == NKC - 1:
                        nc.vector.tensor_scalar_add(
                            dst[m][:, tb * QB : (tb + 1) * QB],
                            st["ps"],
                            b_s[:, m : m + 1],
                        )

                units.append(mk)
            return units

        def proj_v_units(t128):
            tb, i = t128 // (QB // P), t128 % (QB // P)
            units = []
            st = {}
            for kc in range(NKC):

                def mk(kc=kc, st=st, tb=tb, i=i, t128=t128):
                    if kc == 0:
                        st["ps"] = ps_mm.tile(
                            [P, HC], F32, tag="mm", name=f"pz_v{t128}"
                        )
                    nc.tensor.matmul(
                        st["ps"],
                        lhsT=xv_t[tb][:, kc, i * P : (i + 1) * P],
                        rhs=wv_s[:, kc, :],
                        start=(kc == 0),
                        stop=(kc == NKC - 1),
                    )
                    if kc == NKC - 1:
                        nc.vector.tensor_copy(
                            V[:, t128].rearrange("p (h c) -> p h c", c=VW)[:, :, :DK],
                            st["ps"].rearrange("p (h c) -> p h c", c=DK),
                        )

                units.append(mk)
            return units

        def evac_unit(qb, m, h0, h1, pv0, pv1):
            # copy the raw numerator blocks + denominator rows out; DMA the
            # numerator chunk. The host divides and output-projects.
            def mk_evac(qb=qb, m=m, h0=h0, h1=h1, pv0=pv0, pv1=pv1):
                nb = late.tile([P, QB], FP16, name=f"nb_{qb}_{m}")
                nc.vector.tensor_copy(nb[0:DK, :], pv0[:DK, :])
                nc.vector.tensor_copy(nb[DK:P, :], pv1[:DK, :])
                r0, r1 = 4 * qb + h0, 4 * qb + h1
                nc.vector.tensor_copy(
                    dn_s[:, r0 * QB : (r0 + 1) * QB], pv0[DK : DK + 1, :]
                )
                nc.vector.tensor_copy(
                    dn_s[:, r1 * QB : (r1 + 1) * QB], pv1[DK : DK + 1, :]
                )
                nc.sync.dma_start(ac[m, qb], nb)
                if h0 == 2:  # hp1: this q-block's denominators are complete
                    nc.sync.dma_start(
                        dn[4 * qb * QB : (4 * qb + 4) * QB],
                        dn_s[:, 4 * qb * QB : (4 * qb + 4) * QB],
                    )

            return [mk_evac]

        dn_s = late.tile([1, NQB * HL * QB], F32, name="dn_s")

        # ---------------- stage A: minimal prefix ----------------
        proj_qk_direct(xk_t[0], wk_s, bk_s, KT, 0, 0)
        proj_qk_direct(xq_t[0], wq_s, bq_s, QT, 0, 0)

        # Everything else drains as keyed PE filler in just-in-time order.
        def keyed(units, key):
            return [(None, u) for u in units[:-1]] + [(key, units[-1])]

        def kjob(m, tb):
            return keyed(
                proj_qk_units(xk_t[tb], wk_s, bk_s, KT, m, tb, "xk"), ("K", m, tb)
            )

        def qjob(m, tb):
            return keyed(
                proj_qk_units(xq_t[tb], wq_s, bq_s, QT, m, tb, "xq"), ("Q", m, tb)
            )

        def vjob(t128):
            return keyed(proj_v_units(t128), ("V", t128))

        zip_units = (
            vjob(0) + vjob(1) + vjob(2) + vjob(3)
            + kjob(0, 1)
            + vjob(4) + vjob(5) + vjob(6) + vjob(7)
            + kjob(1, 0) + qjob(1, 0)
            + kjob(0, 2)
            + vjob(8) + vjob(9) + vjob(10) + vjob(11)
            + kjob(0, 3)
            + vjob(12) + vjob(13) + vjob(14) + vjob(15)
            + kjob(1, 1) + qjob(0, 1) + qjob(1, 1)
            + kjob(1, 2) + qjob(0, 2)
            + kjob(1, 3) + qjob(1, 2)
            + qjob(0, 3) + qjob(1, 3)
        )
        zq = list(zip_units)[::-1]  # pop from end
        done_keys = {("K", 0, 0), ("Q", 0, 0)}

        def drain(n):
            for _ in range(n):
                if zq:
                    key, fn = zq.pop()
                    fn()
                    if key is not None:
                        done_keys.add(key)

        def drain_until(key):
            while key not in done_keys:
                assert zq, f"drain_until({key}) exhausted the queue"
                drain(1)

        def push_next(units, key=None):
            # zq pops from the end, so append reversed to run these next
            ku = keyed(units, key) if key else [(None, u) for u in units]
            for u in reversed(ku):
                zq.append(u)

        # ---------------- attention ----------------
        # Head pairs (2*hp, 2*hp+1) run their score matmuls concurrently on
        # disjoint PE row groups (K=64 each, base partitions 0 / 64).
        for qb in range(NQB):
            rate = (7, 2, 1, 1)[qb]
            for hp in range(2):
                m = hp  # heads (2*hp, 2*hp+1) live in QT/KT chunk m
                h0, h1 = 2 * hp, 2 * hp + 1
                # the previous head-pair's evac must be EMITTED before this
                # pair's PV matmuls reuse its PSUM slots (in-order queues)
                prev = (qb, 0) if hp == 1 else (qb - 1, 1)
                if prev[0] >= 0:
                    drain_until(("N",) + prev)
                pv0 = ps_pv.tile([VW, QB], F32, tag="pv", name=f"pv_{qb}_{h0}")
                pv1 = ps_pv.tile([VW, QB], F32, tag="pv", name=f"pv_{qb}_{h1}")

                def emit_pv(kb, at, pv0=pv0, pv1=pv1, h0=h0, h1=h1):
                    drain_until(("V", kb))
                    nc.tensor.matmul(
                        pv0,
                        lhsT=V[:, kb, VW * h0 : VW * (h0 + 1)],
                        rhs=at[:, :QB],
                        start=(kb == 0),
                        stop=(kb == NKB - 1),
                    )
                    nc.tensor.matmul(
                        pv1,
                        lhsT=V[:, kb, VW * h1 : VW * (h1 + 1)],
                        rhs=at[:, QB:],
                        start=(kb == 0),
                        stop=(kb == NKB - 1),
                    )

                drain_until(("Q", m, qb))
                # PV trails the scores by 3 kb blocks (elasticity against
                # late V tiles without starving the Scalar exp stream)
                pending = []
                for kb in range(NKB):
                    drain_until(("K", m, kb // 4))
                    sc = ps_sc.tile(
                        [P, 2 * QB], F32, tag="sc", name=f"sc_{qb}_{hp}_{kb}"
                    )
                    nc.tensor.matmul(
                        sc[:, :QB],
                        lhsT=KT[m][0:DK, kb * P : (kb + 1) * P],
                        rhs=QT[m][0:DK, qb * QB : (qb + 1) * QB],
                        start=True,
                        stop=True,
                    )
                    nc.tensor.matmul(
                        sc[:, QB:],
                        lhsT=KT[m][DK:P, kb * P : (kb + 1) * P],
                        rhs=QT[m][DK:P, qb * QB : (qb + 1) * QB],
                        start=True,
                        stop=True,
                    )
                    at = attn_pool.tile(
                        [P, 2 * QB], FP16, tag="at", name=f"at_{qb}_{hp}_{kb}"
                    )
                    nc.scalar.activation(at, sc, AF.Exp, scale=0.125)
                    pending.append((kb, at))
                    if len(pending) > 3:
                        emit_pv(*pending.pop(0))
                    drain(rate)
                for pv_args in pending:
                    emit_pv(*pv_args)

                # raw-numerator evacuation runs as filler after the next
                # head-pair's first scores
                push_next(
                    evac_unit(qb, m, h0, h1, pv0, pv1), key=("N", qb, hp)
                )

        drain(10_000)


_module_cache = None


def get_module():
    global _module_cache
    if _module_cache is None:
        _module_cache = build_module()
    return _module_cache


def _pack_x(xT_f16):
    # [D, T] fp16 -> [NQB, P, NKC, QB]: tb-block, partition-major, contiguous
    return np.ascontiguousarray(
        xT_f16.reshape(NKC, P, NQB, QB).transpose(2, 1, 0, 3)
    )


def shard_inputs(query, key, value, Wq, bq, Wk, bk, Wv, bv, Wo, bo):
    """Build the 8 per-core input maps (host-side layout transforms only)."""
    f = np.float32
    h = np.float16
    xP = {}
    for b in range(B):
        xP["q", b] = _pack_x(np.asarray(query, f)[:, b, :].T.astype(h))
        xP["k", b] = _pack_x(np.asarray(key, f)[:, b, :].T.astype(h))
        xP["v", b] = _pack_x(np.asarray(value, f)[:, b, :].T.astype(h))
    Wq, Wk, Wv = (np.asarray(w, f) for w in (Wq, Wk, Wv))
    bq, bk = np.asarray(bq, f), np.asarray(bk, f)

    def pack_w(Wcols):  # [HC, D] rows=outcols -> [P, NKC, HC]
        return np.ascontiguousarray(
            Wcols.T.astype(h).reshape(NKC, P, HC).transpose(1, 0, 2)
        )

    def pack_w_m(Wcols):  # [HC, D] -> [2, P, NKC, P] (per 128-col m chunk)
        return np.ascontiguousarray(
            Wcols.T.astype(h).reshape(NKC, P, 2, P).transpose(2, 1, 0, 3)
        )

    in_maps = []
    for c in range(NCORES):
        b, hg = c // (NCORES // B), c % (NCORES // B)
        cols = slice(HC * hg, HC * (hg + 1))
        in_maps.append(
            {
                "xq": xP["q", b],
                "xk": xP["k", b],
                "xv": xP["v", b],
                "wq": pack_w_m(Wq[cols, :]),
                "wk": pack_w_m(Wk[cols, :]),
                "wv": pack_w(Wv[cols, :]),
                "bqv": np.ascontiguousarray(
                    bq[cols].reshape(HC // P, P).T.astype(f)
                ),
                "bkv": np.ascontiguousarray(
                    bk[cols].reshape(HC // P, P).T.astype(f)
                ),
            }
        )
    return in_maps


def kernel(query, key, value, Wq, bq, Wk, bk, Wv, bv, Wo, bo, trace=False):
    nc = get_module()
    in_maps = shard_inputs(query, key, value, Wq, bq, Wk, bk, Wv, bv, Wo, bo)
    res = bass_utils.run_bass_kernel_spmd(
        nc, in_maps, core_ids=list(range(NCORES)), trace=trace
    )
    f = np.float32
    Wo = np.asarray(Wo, f)
    bias_term = np.asarray(bv, f) @ Wo.T + np.asarray(bo, f)
    output = np.empty((S, B, D), f)
    for b in range(B):
        acc = None
        for c in range(4 * b, 4 * b + 4):
            hg = c % 4
            cols = slice(HC * hg, HC * (hg + 1))
            acr = res.results[c]["ac"].astype(f)   # [2, NQB, P, QB]
            dnr = res.results[c]["dn"].astype(f).reshape(NQB * HL, QB)
            # A[m] is [128, 2048]: feature-major numerators for heads 2m,2m+1
            A = acr.transpose(0, 2, 1, 3).reshape(2, P, T)
            # divide each head's 64-row block by its (qb, h) denominator
            for m in range(2):
                for hh in range(2):
                    hloc = 2 * m + hh
                    off = 64 * hh
                    den = dnr.reshape(NQB, HL, QB)[:, hloc, :].reshape(T)
                    A[m, off : off + DK, :] /= den[None, :]
            # partial output projection for this core's 256 features
            Afull = A.reshape(HC, T)              # [256, 2048]
            part = Afull.T @ Wo[:, cols].T.astype(f)  # [2048, 1024]
            acc = part if acc is None else acc + part
        output[:, b, :] = acc + bias_term
    if trace:
        kernel.last_results = res
    return output



# revision 7
# speedup vs baseline: 1.1030x; 1.1030x over previous
"""MultiHeadAttention TRN2 Bass kernel (v6).

Problem: S=2048, B=2, H=16, d_k=64, D=1024, fp32.
  q = query @ Wq.T + bq ; k = key @ Wk.T + bk ; v = value @ Wv.T + bv
  score = einsum('qbhd,kbhd->qkbh', q, k) / 8 ; attn = softmax(score, axis=k)
  out = einsum('qkbh,kbhd->qbhd', attn, v) -> reshape -> @ Wo.T + bo

Sharding (8 cores): core c handles batch b = c//4 and heads [4*(c%4), 4*(c%4)+4)
(tensor-parallel along the head dimension). The device computes the QKV
projections and the attention (scores -> exp -> PV with a fused ones-column
denominator). Each core returns its raw PV numerators [256, 2048] plus the
16 softmax denominator rows; the gather/unshard step divides and applies
the output projection while it sums the per-core partials and bias terms.

v6: the v5 kernel was Scalar-engine bound (128 exp ACTIVATEs of [128,1024]
at ~1.35us each = 172us > the PE's ~123us of matmul work). v6 splits the
exp stream across TWO engines: even kb-blocks keep the exact ACT exp; odd
kb-blocks run a custom single-instruction DVE op (EXP2_FAST_ANT) that
computes exp via the int16-bitcast trick with an |frac| linear correction
(max rel err 0.88%, end-to-end attention err ~6e-3, budget 2e-2). The two
engines work adjacent sc PSUM slots concurrently, halving the exp wall
time and leaving the PE (scores + PV + projections) as the bottleneck.
Projection bias-adds move DVE->ACT (Identity activation with bias AP,
same act table set as Exp) to balance the two exp engines.

Device schedule (see v3/v4/v5 notes):
  - All matmul operands fp16; fp32 PSUM accumulate.
  - Host passes tensors pre-packed in SBUF layout (multi-KB descriptors);
    x tiles stream over the Sync HWDGE queue in just-in-time order, weights
    over the Activation HWDGE queue at kernel start.
  - A few dummy matmuls ramp the PE DVFS p-state during the DMA prefix.
  - PV matmuls trail the score matmuls by 3 kb-blocks; projections drain as
    keyed PE filler between attention matmuls (forced drains keep producers
    ahead of consumers in the in-order engine queues).
"""

import os

os.environ.setdefault("MYCRO_LOCAL_CACHE", "1")

import numpy as np

import concourse.bass as bass
import concourse.tile as tile
from concourse import bacc, bass_utils, mybir


def _install_ntff_hook():
    """Provide antenv.axon_hooks when the image lacks it, so trace=True can
    capture NTFF profiles through the axon tunnel. Degrades silently."""
    import contextlib
    import ctypes
    import sys

    if "antenv.axon_hooks" in sys.modules:
        return
    so_path = "/opt/axon/libaxon_pjrt.so"
    if not os.path.exists(so_path):
        return
    try:
        lib = ctypes.CDLL(so_path)
        if not hasattr(lib, "axon_start_nrt_profile"):
            return
        lib.axon_start_nrt_profile.argtypes = [
            ctypes.POINTER(ctypes.c_int64),
            ctypes.c_size_t,
        ]
        lib.axon_start_nrt_profile.restype = ctypes.c_int64
        lib.axon_stop_nrt_profile.argtypes = [ctypes.c_char_p]
        lib.axon_stop_nrt_profile.restype = ctypes.c_int64

        @contextlib.contextmanager
        def _hook(output_dir, device_ids):
            import jax

            jax.devices()
            if device_ids:
                ids = (ctypes.c_int64 * len(device_ids))(*device_ids)
                rc = lib.axon_start_nrt_profile(ids, len(device_ids))
            else:
                rc = lib.axon_start_nrt_profile(None, 0)
            if rc != 0:
                raise RuntimeError(f"axon_start_nrt_profile rc={rc}")
            try:
                yield
            finally:
                n = lib.axon_stop_nrt_profile(str(output_dir).encode())
                print(f"ntff profile: {n} file(s) -> {output_dir}")

        import types

        mod = types.ModuleType("antenv.axon_hooks")
        mod.get_axon_ntff_profile_hook = lambda: _hook
        mod.set_axon_ntff_profile_hook = lambda h: None
        sys.modules["antenv.axon_hooks"] = mod
    except Exception:
        pass


_install_ntff_hook()

F32 = mybir.dt.float32
FP16 = mybir.dt.float16
I16 = mybir.dt.int16
AF = mybir.ActivationFunctionType

# ---------------------------------------------------------------------------
# Custom DVE op: fast exp16 via the int16-bitcast trick.
#
#   y = score * (0.125*log2(e)*1024)            (1024-scaled base-2 exponent)
#   r = (y + 1.5*2^33) - 1.5*2^33               (fp32 magic-add: round y to a
#                                                multiple of 1024)
#   v = y + |y - r| * A + K                     (|frac| linear mantissa fix)
#   at16 = bitcast_fp16(int16(v))               (output-stage conversion
#                                                assembles exponent+mantissa)
#
# A, K minimax-fit: max rel err 0.88%, rms 0.47%. The DVE has no exp; this
# runs at 1 elem/cycle/lane as a single instruction (2 uops, 7 ALU stages),
# letting the Vector engine take half the softmax exp stream off the
# Scalar engine. Registered via the documented dve_ops extension pattern
# (04-custom-dve-api.md: "define a DveOp constant and append it to OPS").
# ---------------------------------------------------------------------------
EXP2_C0 = 0.125 * float(np.log2(np.e)) * 1024.0   # score -> 1024*log2 units
EXP2_MAGIC = 1.5 * 2.0**33                         # fp32 round-to-1024 magic
EXP2_A = -0.175477                                 # |frac| slope correction
EXP2_K = 15349.7375                                # exponent bias + offset


def _register_exp2_op():
    import concourse.dve_ops as dve_ops_mod
    from concourse.dve_spec import Spec, Src0, C0, C1, C2, C3, AluOp, Bin, lower
    from concourse.dve_spec import _has_src1
    from concourse.dve_uop import DveOpSpec

    if any(op.name == "EXP2_FAST_ANT" for op in dve_ops_mod.OPS):
        return next(op for op in dve_ops_mod.OPS if op.name == "EXP2_FAST_ANT")

    y = Src0 * C0
    u = y + C1
    r = u - C1
    b = Bin(AluOp.ABSOLUTE_DIFF, y, r)
    body = dve_ops_mod._spill_c3_to_src1((y + (b * C2)) + C3)

    def _ref_exp16(in0, in1, s0, s1, imm2):
        f32 = np.float32
        y = (in0.astype(f32) * f32(s0)).astype(f32)
        u = (y + f32(s1)).astype(f32)
        r = (u - f32(s1)).astype(f32)
        b = np.abs((y - r).astype(f32))
        v = (y + (b * f32(imm2)).astype(f32)).astype(f32)
        return v + np.asarray(in1, f32).reshape(-1, 1)

    spec = Spec(body=body, reference=_ref_exp16)
    shas = {}
    for ver in ("v3", "v4"):
        uops = lower(spec, ver=ver)
        shas[ver] = DveOpSpec(
            name="EXP2_FAST_ANT", opcode=0, uops=uops, rd1_en=_has_src1(spec)
        ).sha(ver)
    op = dve_ops_mod.DveOp("EXP2_FAST_ANT", spec, subdim=False, uops_sha=shas)
    dve_ops_mod.OPS.append(op)
    dve_ops_mod._SUB_OPCODE_FOR_NAME[op.name] = (
        dve_ops_mod._CUSTOM_DVE_ROW_BASE + len(dve_ops_mod.OPS) - 1
    )
    dve_ops_mod.CUSTOM_DVE_SPECS[op.name] = op.spec
    return op


EXP2_FAST_ANT = _register_exp2_op()

S = 2048          # sequence length
B = 2             # batch
H = 16            # total heads
DK = 64           # head dim
D = 1024          # model dim
NCORES = 8
HL = H // (NCORES // B)   # heads per core = 4
HC = HL * DK              # head cols per core = 256
T = S                     # tokens per core (one batch element)
P = 128
QB = 512                  # q block (matmul free dim)
NKB = T // P              # 16 k blocks
NQB = T // QB             # 4 q blocks
NKC = D // P              # 8 contraction chunks for projections
VW = DK + 1               # 65: head value cols + ones column


def build_module():
    nc = bacc.Bacc("TRN2", target_bir_lowering=False, debug=False)

    # Inputs pre-packed by the host in SBUF layout (partition-major,
    # contiguous per partition -> multi-KB DMA descriptors).
    xq = nc.dram_tensor("xq", [NQB, P, NKC, QB], FP16, kind="ExternalInput").ap()
    xk = nc.dram_tensor("xk", [NQB, P, NKC, QB], FP16, kind="ExternalInput").ap()
    xv = nc.dram_tensor("xv", [NQB, P, NKC, QB], FP16, kind="ExternalInput").ap()
    wq = nc.dram_tensor("wq", [2, P, NKC, P], FP16, kind="ExternalInput").ap()
    wk = nc.dram_tensor("wk", [2, P, NKC, P], FP16, kind="ExternalInput").ap()
    wv = nc.dram_tensor("wv", [P, NKC, HC], FP16, kind="ExternalInput").ap()
    bqv = nc.dram_tensor("bqv", [P, HC // P], F32, kind="ExternalInput").ap()
    bkv = nc.dram_tensor("bkv", [P, HC // P], F32, kind="ExternalInput").ap()
    # raw attention numerators [m, qb, 128, 512] and denominators
    # (flat [ (4qb+h)*512 + col ] on one partition: engines may not write
    # single-partition tiles at arbitrary partition offsets)
    ac = nc.dram_tensor("ac", [2, NQB, P, QB], FP16, kind="ExternalOutput").ap()
    dn = nc.dram_tensor("dn", [NQB * HL * QB], F32, kind="ExternalOutput").ap()

    with tile.TileContext(nc) as tc:
        kernel_body(tc, xq, xk, xv, wq, wk, wv, bqv, bkv, ac, dn)

    nc.compile()
    return nc


def kernel_body(tc, xq, xk, xv, wq, wk, wv, bqv, bkv, ac, dn):
    nc = tc.nc

    with (
        tc.tile_pool(name="attn", bufs=8) as attn_pool,
        tc.tile_pool(name="consts", bufs=1) as consts,
        tc.tile_pool(name="persist", bufs=1) as persist,
        tc.tile_pool(name="late", bufs=1) as late,
        tc.tile_pool(name="ps_mm", bufs=2, space="PSUM") as ps_mm,
        tc.tile_pool(name="ps_sc", bufs=2, space="PSUM") as ps_sc,
        tc.tile_pool(name="ps_pv", bufs=2, space="PSUM") as ps_pv,
    ):
        # at tiles first: the Activation engine's SBUF write latency grows
        # with address, and the 128 exps are the kernel's critical path.
        at_tiles = [
            attn_pool.tile([P, 2 * QB], FP16, tag="at", name=f"at_{i}")
            for i in range(8)
        ]
        # ---------------- PE warm-up (ramps the DVFS p-state) --------------
        dummy = consts.tile([1, QB], FP16)
        nc.vector.memset(dummy, 1.0)
        warm_ps = ps_mm.tile([DK, QB], F32, tag="mm", name="warm")
        for _ in range(6):
            nc.tensor.matmul(
                warm_ps, lhsT=dummy[:, :DK], rhs=dummy, start=True, stop=True
            )

        # ---------------- inputs: three parallel DMA queues ----------------
        # Each hardware/software queue streams ~1MB/8us, so K, V and Q
        # traffic ride separate queues, ordered just-in-time. wk/wq are
        # split per 128-col m-chunk so stage A waits on only 0.25MB.
        xk_t = [persist.tile([P, NKC, QB], FP16, name=f"xk{tb}") for tb in range(4)]
        xv_t = [persist.tile([P, NKC, QB], FP16, name=f"xv{tb}") for tb in range(4)]
        xq_t = [persist.tile([P, NKC, QB], FP16, name=f"xq{tb}") for tb in range(4)]
        wk_s = [consts.tile([P, NKC, P], FP16, name=f"wk_s{m}") for m in range(2)]
        wq_s = [consts.tile([P, NKC, P], FP16, name=f"wq_s{m}") for m in range(2)]
        wv_s = consts.tile([P, NKC, HC], FP16)
        bk_s = consts.tile([P, HC // P], F32)
        bq_s = consts.tile([P, HC // P], F32)

        # Sync queue: all x inputs just-in-time (one queue sustains ~1MB/9us;
        # the ACT queue moves xq0 slower than sync even under contention)
        for dst, src, tb in (
            (xk_t[0], xk, 0), (xq_t[0], xq, 0),
            (xv_t[0], xv, 0), (xv_t[1], xv, 1),
            (xk_t[1], xk, 1), (xv_t[2], xv, 2),
            (xk_t[2], xk, 2), (xv_t[3], xv, 3),
            (xk_t[3], xk, 3),
            (xq_t[1], xq, 1), (xq_t[2], xq, 2), (xq_t[3], xq, 3),
        ):
            nc.sync.dma_start(dst, src[tb])
        # Activation HWDGE queue: the small weight/bias chunks in parallel
        nc.scalar.dma_start(wk_s[0], wk[0])
        nc.scalar.dma_start(wq_s[0], wq[0])
        nc.scalar.dma_start(wk_s[1], wk[1])
        nc.scalar.dma_start(wq_s[1], wq[1])
        nc.scalar.dma_start(bq_s, bqv)
        nc.scalar.dma_start(bk_s, bkv)
        # GpSimd queue: only the V weights (bulk x traffic on this software
        # queue slows the whole DMA fabric — measured +15us)
        nc.gpsimd.dma_start(wv_s, wv)

        ones_f32 = consts.tile([P, DK], F32)
        nc.vector.memset(ones_f32, 1.0)
        # per-partition K constant for the custom DVE exp (C3 spilled to Src1)
        kconst = consts.tile([P, 1], F32)
        nc.vector.memset(kconst, EXP2_K)

        # ---------------- persistent activations ----------------
        QT = [persist.tile([P, T], FP16, name=f"QT{m}") for m in range(2)]
        KT = [persist.tile([P, T], FP16, name=f"KT{m}") for m in range(2)]
        V = persist.tile([P, NKB, HL * VW], FP16, name="V")

        # ones columns of V (denominator trick); also warms the act table
        nc.scalar.activation(
            V.rearrange("p t (h c) -> p t h c", c=VW)[:, :, :, DK],
            ones_f32[:, : NKB * HL].rearrange("p (t h) -> p t h", h=HL),
            AF.Copy,
        )

        # ---------------- projection emitters (merged drain units) ---------
        def proj_qk_direct(xt, w_s, b_s, dst, m, tb):
            ps = ps_mm.tile([P, QB], F32, tag="mm", name=f"pd_{dst[0].name}{m}{tb}")
            for kc in range(NKC):
                nc.tensor.matmul(
                    ps,
                    lhsT=w_s[m][:, kc, :],
                    rhs=xt[:, kc, :],
                    start=(kc == 0),
                    stop=(kc == NKC - 1),
                )
            nc.scalar.activation(
                dst[m][:, tb * QB : (tb + 1) * QB],
                ps,
                AF.Identity,
                bias=b_s[:, m : m + 1],
            )

        def proj_qk_units(xt, w_s, b_s, dst, m, tb, tag):
            # 8 units: [alloc+mm0], mm1..mm6, [mm7+evac]
            units = []
            st = {}
            for kc in range(NKC):

                def mk(kc=kc, st=st):
                    if kc == 0:
                        st["ps"] = ps_mm.tile(
                            [P, QB], F32, tag="mm", name=f"pz_{tag}{m}{tb}"
                        )
                    nc.tensor.matmul(
                        st["ps"],
                        lhsT=w_s[m][:, kc, :],
                        rhs=xt[:, kc, :],
                        start=(kc == 0),
                        stop=(kc == NKC - 1),
                    )
                    if kc == NKC - 1:
                        nc.scalar.activation(
                            dst[m][:, tb * QB : (tb + 1) * QB],
                            st["ps"],
                            AF.Identity,
                            bias=b_s[:, m : m + 1],
                        )

                units.append(mk)
            return units

        def proj_v_units(t128):
            tb, i = t128 // (QB // P), t128 % (QB // P)
            units = []
            st = {}
            for kc in range(NKC):

                def mk(kc=kc, st=st, tb=tb, i=i, t128=t128):
                    if kc == 0:
                        st["ps"] = ps_mm.tile(
                            [P, HC], F32, tag="mm", name=f"pz_v{t128}"
                        )
                    nc.tensor.matmul(
                        st["ps"],
                        lhsT=xv_t[tb][:, kc, i * P : (i + 1) * P],
                        rhs=wv_s[:, kc, :],
                        start=(kc == 0),
                        stop=(kc == NKC - 1),
                    )
                    if kc == NKC - 1:
                        nc.vector.tensor_copy(
                            V[:, t128].rearrange("p (h c) -> p h c", c=VW)[:, :, :DK],
                            st["ps"].rearrange("p (h c) -> p h c", c=DK),
                        )

                units.append(mk)
            return units

        def evac_unit(qb, m, h0, h1, pv0, pv1):
            # copy the raw numerator blocks + denominator rows out; DMA the
            # numerator chunk. The host divides and output-projects.
            def mk_evac(qb=qb, m=m, h0=h0, h1=h1, pv0=pv0, pv1=pv1):
                nb = late.tile([P, QB], FP16, name=f"nb_{qb}_{m}")
                nc.vector.tensor_copy(nb[0:DK, :], pv0[:DK, :])
                nc.vector.tensor_copy(nb[DK:P, :], pv1[:DK, :])
                r0, r1 = 4 * qb + h0, 4 * qb + h1
                nc.vector.tensor_copy(
                    dn_s[:, r0 * QB : (r0 + 1) * QB], pv0[DK : DK + 1, :]
                )
                nc.vector.tensor_copy(
                    dn_s[:, r1 * QB : (r1 + 1) * QB], pv1[DK : DK + 1, :]
                )
                nc.sync.dma_start(ac[m, qb], nb)
                if h0 == 2:  # hp1: this q-block's denominators are complete
                    nc.sync.dma_start(
                        dn[4 * qb * QB : (4 * qb + 4) * QB],
                        dn_s[:, 4 * qb * QB : (4 * qb + 4) * QB],
                    )

            return [mk_evac]

        dn_s = late.tile([1, NQB * HL * QB], F32, name="dn_s")

        # ---------------- stage A: minimal prefix ----------------
        proj_qk_direct(xk_t[0], wk_s, bk_s, KT, 0, 0)
        proj_qk_direct(xq_t[0], wq_s, bq_s, QT, 0, 0)

        # Everything else drains as keyed PE filler in just-in-time order.
        def keyed(units, key):
            return [(None, u) for u in units[:-1]] + [(key, units[-1])]

        def kjob(m, tb):
            return keyed(
                proj_qk_units(xk_t[tb], wk_s, bk_s, KT, m, tb, "xk"), ("K", m, tb)
            )

        def qjob(m, tb):
            return keyed(
                proj_qk_units(xq_t[tb], wq_s, bq_s, QT, m, tb, "xq"), ("Q", m, tb)
            )

        def vjob(t128):
            return keyed(proj_v_units(t128), ("V", t128))

        zip_units = (
            vjob(0) + vjob(1) + vjob(2) + vjob(3)
            + kjob(0, 1)
            + vjob(4) + vjob(5) + vjob(6) + vjob(7)
            + kjob(1, 0) + qjob(1, 0)
            + kjob(0, 2)
            + vjob(8) + vjob(9) + vjob(10) + vjob(11)
            + kjob(0, 3)
            + vjob(12) + vjob(13) + vjob(14) + vjob(15)
            + kjob(1, 1) + qjob(0, 1) + qjob(1, 1)
            + kjob(1, 2) + qjob(0, 2)
            + kjob(1, 3) + qjob(1, 2)
            + qjob(0, 3) + qjob(1, 3)
        )
        zq = list(zip_units)[::-1]  # pop from end
        done_keys = {("K", 0, 0), ("Q", 0, 0)}

        def drain(n):
            for _ in range(n):
                if zq:
                    key, fn = zq.pop()
                    fn()
                    if key is not None:
                        done_keys.add(key)

        def drain_until(key):
            while key not in done_keys:
                assert zq, f"drain_until({key}) exhausted the queue"
                drain(1)

        def push_next(units, key=None):
            # zq pops from the end, so append reversed to run these next
            ku = keyed(units, key) if key else [(None, u) for u in units]
            for u in reversed(ku):
                zq.append(u)

        # ---------------- attention ----------------
        # Head pairs (2*hp, 2*hp+1) run their score matmuls concurrently on
        # disjoint PE row groups (K=64 each, base partitions 0 / 64).
        for qb in range(NQB):
            rate = (7, 2, 1, 1)[qb]
            for hp in range(2):
                m = hp  # heads (2*hp, 2*hp+1) live in QT/KT chunk m
                h0, h1 = 2 * hp, 2 * hp + 1
                # the previous head-pair's evac must be EMITTED before this
                # pair's PV matmuls reuse its PSUM slots (in-order queues)
                prev = (qb, 0) if hp == 1 else (qb - 1, 1)
                if prev[0] >= 0:
                    drain_until(("N",) + prev)
                pv0 = ps_pv.tile([VW, QB], F32, tag="pv", name=f"pv_{qb}_{h0}")
                pv1 = ps_pv.tile([VW, QB], F32, tag="pv", name=f"pv_{qb}_{h1}")

                def emit_pv(kb, at, pv0=pv0, pv1=pv1, h0=h0, h1=h1):
                    drain_until(("V", kb))
                    nc.tensor.matmul(
                        pv0,
                        lhsT=V[:, kb, VW * h0 : VW * (h0 + 1)],
                        rhs=at[:, :QB],
                        start=(kb == 0),
                        stop=(kb == NKB - 1),
                    )
                    nc.tensor.matmul(
                        pv1,
                        lhsT=V[:, kb, VW * h1 : VW * (h1 + 1)],
                        rhs=at[:, QB:],
                        start=(kb == 0),
                        stop=(kb == NKB - 1),
                    )

                drain_until(("Q", m, qb))
                # PV trails the scores by 3 kb blocks (elasticity against
                # late V tiles without starving the Scalar exp stream)
                pending = []
                for kb in range(NKB):
                    drain_until(("K", m, kb // 4))
                    sc = ps_sc.tile(
                        [P, 2 * QB], F32, tag="sc", name=f"sc_{qb}_{hp}_{kb}"
                    )
                    nc.tensor.matmul(
                        sc[:, :QB],
                        lhsT=KT[m][0:DK, kb * P : (kb + 1) * P],
                        rhs=QT[m][0:DK, qb * QB : (qb + 1) * QB],
                        start=True,
                        stop=True,
                    )
                    nc.tensor.matmul(
                        sc[:, QB:],
                        lhsT=KT[m][DK:P, kb * P : (kb + 1) * P],
                        rhs=QT[m][DK:P, qb * QB : (qb + 1) * QB],
                        start=True,
                        stop=True,
                    )
                    at = attn_pool.tile(
                        [P, 2 * QB], FP16, tag="at", name=f"at_{qb}_{hp}_{kb}"
                    )
                    if kb % 2 == 1:
                        # odd blocks: custom DVE fast exp (int16-bitcast)
                        nc.vector._custom_dve(
                            EXP2_FAST_ANT,
                            out=at[:, :].bitcast(I16),
                            in0=sc[:, :],
                            in1=kconst[:, :],
                            s0=EXP2_C0,
                            s1=EXP2_MAGIC,
                            imm2=EXP2_A,
                        )
                    else:
                        # even blocks: exact exp on the Scalar engine
                        nc.scalar.activation(at, sc, AF.Exp, scale=0.125)
                    pending.append((kb, at))
                    if len(pending) > 3:
                        emit_pv(*pending.pop(0))
                    drain(rate)
                for pv_args in pending:
                    emit_pv(*pv_args)

                # raw-numerator evacuation runs as filler after the next
                # head-pair's first scores
                push_next(
                    evac_unit(qb, m, h0, h1, pv0, pv1), key=("N", qb, hp)
                )

        drain(10_000)


_module_cache = None


def get_module():
    global _module_cache
    if _module_cache is None:
        _module_cache = build_module()
    return _module_cache


def _pack_x(xT_f16):
    # [D, T] fp16 -> [NQB, P, NKC, QB]: tb-block, partition-major, contiguous
    return np.ascontiguousarray(
        xT_f16.reshape(NKC, P, NQB, QB).transpose(2, 1, 0, 3)
    )


def shard_inputs(query, key, value, Wq, bq, Wk, bk, Wv, bv, Wo, bo):
    """Build the 8 per-core input maps (host-side layout transforms only)."""
    f = np.float32
    h = np.float16
    xP = {}
    for b in range(B):
        xP["q", b] = _pack_x(np.asarray(query, f)[:, b, :].T.astype(h))
        xP["k", b] = _pack_x(np.asarray(key, f)[:, b, :].T.astype(h))
        xP["v", b] = _pack_x(np.asarray(value, f)[:, b, :].T.astype(h))
    Wq, Wk, Wv = (np.asarray(w, f) for w in (Wq, Wk, Wv))
    bq, bk = np.asarray(bq, f), np.asarray(bk, f)

    def pack_w(Wcols):  # [HC, D] rows=outcols -> [P, NKC, HC]
        return np.ascontiguousarray(
            Wcols.T.astype(h).reshape(NKC, P, HC).transpose(1, 0, 2)
        )

    def pack_w_m(Wcols):  # [HC, D] -> [2, P, NKC, P] (per 128-col m chunk)
        return np.ascontiguousarray(
            Wcols.T.astype(h).reshape(NKC, P, 2, P).transpose(2, 1, 0, 3)
        )

    in_maps = []
    for c in range(NCORES):
        b, hg = c // (NCORES // B), c % (NCORES // B)
        cols = slice(HC * hg, HC * (hg + 1))
        in_maps.append(
            {
                "xq": xP["q", b],
                "xk": xP["k", b],
                "xv": xP["v", b],
                "wq": pack_w_m(Wq[cols, :]),
                "wk": pack_w_m(Wk[cols, :]),
                "wv": pack_w(Wv[cols, :]),
                "bqv": np.ascontiguousarray(
                    bq[cols].reshape(HC // P, P).T.astype(f)
                ),
                "bkv": np.ascontiguousarray(
                    bk[cols].reshape(HC // P, P).T.astype(f)
                ),
            }
        )
    return in_maps


def kernel(query, key, value, Wq, bq, Wk, bk, Wv, bv, Wo, bo, trace=False):
    nc = get_module()
    in_maps = shard_inputs(query, key, value, Wq, bq, Wk, bk, Wv, bv, Wo, bo)
    res = bass_utils.run_bass_kernel_spmd(
        nc, in_maps, core_ids=list(range(NCORES)), trace=trace
    )
    f = np.float32
    Wo = np.asarray(Wo, f)
    bias_term = np.asarray(bv, f) @ Wo.T + np.asarray(bo, f)
    output = np.empty((S, B, D), f)
    for b in range(B):
        acc = None
        for c in range(4 * b, 4 * b + 4):
            hg = c % 4
            cols = slice(HC * hg, HC * (hg + 1))
            acr = res.results[c]["ac"].astype(f)   # [2, NQB, P, QB]
            dnr = res.results[c]["dn"].astype(f).reshape(NQB * HL, QB)
            # A[m] is [128, 2048]: feature-major numerators for heads 2m,2m+1
            A = acr.transpose(0, 2, 1, 3).reshape(2, P, T)
            # divide each head's 64-row block by its (qb, h) denominator
            for m in range(2):
                for hh in range(2):
                    hloc = 2 * m + hh
                    off = 64 * hh
                    den = dnr.reshape(NQB, HL, QB)[:, hloc, :].reshape(T)
                    A[m, off : off + DK, :] /= den[None, :]
            # partial output projection for this core's 256 features
            Afull = A.reshape(HC, T)              # [256, 2048]
            part = Afull.T @ Wo[:, cols].T.astype(f)  # [2048, 1024]
            acc = part if acc is None else acc + part
        output[:, b, :] = acc + bias_term
    if trace:
        kernel.last_results = res
    return output

